# revision 10
# baseline (speedup 1.0000x reference)
"""Distributed GQA attention (B=2, S=2048, D=2048, H=32, KVH=8, HD=64,
causal + interleaved RoPE) on 8 Trainium2 NeuronCores.

Sharding (uniform SPMD -- one program, zero divergent control flow):
  Core c owns q-heads [4c, 4c+4) == exactly kv-head c, for BOTH batches.
  Two 8-core AllToAlls (bf16, 1MB each) re-shard the attention output from
  head-split to seq-split; after the A2As each core holds attn^T[all 2048
  features, its 512 q rows] and emits the FINAL out^T slice.

v2 structure (from perfetto/NTFF analysis of the 687us baseline):
  - QK for the two heads of a qT tile are emitted back-to-back with
    row-disjoint tile positions (rows 0:64 / 64:128) so the PE runs them
    CONCURRENTLY -- the attention QK stream halves.
  - exp runs as one [128,1024] ACTIVATE per kc (2 PSUM banks) instead of
    two [128,512] -- halves the 352-cycle per-instruction ACT overhead.
  - causal mask adds shrink from [128,512] to the [128,128] triangle
    block; fully-masked columns are skipped in QK and zeroed in pt.
  - softmax normalize: reciprocal on [1,512] rows directly, one DRAM hop
    for the partition-broadcast (was 3).
  - scalar queue carries ONLY activations (DMAs moved to sync/gpsimd) --
    the baseline's 140us of scalar-queue DMA starved exp and stalled the
    PE on PSUM WAR at the A2A windows.
  - x loads split into 512KB (g,j) chunks so projections chase the DMA.
"""
import sys
if '/opt/trn_rl_repo' not in sys.path:
    sys.path.insert(0, '/opt/trn_rl_repo')

import numpy as np
import ml_dtypes
from contextlib import ExitStack

import concourse.bass as bass
import concourse.bacc as bacc
import concourse.tile as tile
from concourse import mybir
from concourse.bass_utils import run_bass_kernel_spmd

B, S, D = 2, 2048, 2048
H, KVH, HD = 32, 8, 64
NCORES = 8
BF16_MIN = -3.3895313892515355e+38
BF16 = mybir.dt.bfloat16
F32 = mybir.dt.float32
BD = ml_dtypes.bfloat16

_CACHE = {}


def _build():
    nc = bacc.Bacc("TRN2", target_bir_lowering=False, debug=False,
                   num_devices=NCORES, name="attn")

    xkv_e = nc.declare_dram_parameter("xkv", [2, 4, 128, 4, S], BF16, False)   # [b, g, p, j, s]
    wq_e = nc.declare_dram_parameter("wqt", [2, 16, 128, 128], BF16, False)    # [t, dk, d, e]
    wk_e = nc.declare_dram_parameter("wkt", [16, 128, 64], BF16, False)
    wv_e = nc.declare_dram_parameter("wvt", [16, 128, 64], BF16, False)
    wo_e = nc.declare_dram_parameter("wot", [4, 16, 128, 512], BF16, False)    # [dcg, ec, e, d]
    ropec_e = nc.declare_dram_parameter("ropec", [128, S], F32, False)
    ropeg_e = nc.declare_dram_parameter("ropeg", [128, S], F32, False)
    perm_e = nc.declare_dram_parameter("perm", [128, 128], F32, False)
    mask_e = nc.declare_dram_parameter("mask128", [128, 128], BF16, False)
    out_e = nc.declare_dram_parameter("out", [D, 512], F32, True)

    with tile.TileContext(nc) as tc, ExitStack() as ctx:
        xkv_p = ctx.enter_context(tc.tile_pool(name="xkv", bufs=4))
        res_p = ctx.enter_context(tc.tile_pool(name="res", bufs=1))
        scr_p = ctx.enter_context(tc.tile_pool(name="scr", bufs=2))
        pt_p = ctx.enter_context(tc.tile_pool(name="pt", bufs=6))
        nrm_p = ctx.enter_context(tc.tile_pool(name="nrm", bufs=4))
        oev_p = ctx.enter_context(tc.tile_pool(name="oev", bufs=4))
        wst_p = ctx.enter_context(tc.tile_pool(name="wst", bufs=12))
        vpl_p = ctx.enter_context(tc.tile_pool(name="vpl", bufs=32))
        dram_p = ctx.enter_context(tc.tile_pool(name="dram", bufs=1, space="DRAM"))
        # PSUM: 8 banks total.  qk quads [128,1024]x2 = 4, po [65,512]x3 = 3,
        # op (outproj / rope-pp) [128,512]x1 = 1.
        psQ = ctx.enter_context(tc.tile_pool(name="psQ", bufs=2, space="PSUM"))
        psPV = ctx.enter_context(tc.tile_pool(name="psPV", bufs=3, space="PSUM"))
        psOP = ctx.enter_context(tc.tile_pool(name="psOP", bufs=1, space="PSUM"))

        # ---- resident constants: weights on gpsimd queue, x on sync ----
        wk_t, wv_t, wq_t = [], [], {}
        for dk in range(16):
            a = res_p.tile([128, 64], BF16, tag=f"wk{dk}", name=f"wk{dk}")
            nc.gpsimd.dma_start(out=a[:], in_=wk_e[dk])
            wk_t.append(a)
        ropec = res_p.tile([128, S], F32)
        ropeg = res_p.tile([128, S], F32)
        permt = res_p.tile([128, 128], F32)
        nc.gpsimd.dma_start(out=ropec[:], in_=ropec_e[:])
        nc.gpsimd.dma_start(out=ropeg[:], in_=ropeg_e[:])
        nc.gpsimd.dma_start(out=permt[:], in_=perm_e[:])
        for dk in range(16):
            bb = res_p.tile([128, 64], BF16, tag=f"wv{dk}", name=f"wv{dk}")
            nc.gpsimd.dma_start(out=bb[:], in_=wv_e[dk])
            wv_t.append(bb)
        for t in range(2):
            for dk in range(16):
                w = res_p.tile([128, 128], BF16, tag=f"wq{t}_{dk}", name=f"wq{t}_{dk}")
                nc.gpsimd.dma_start(out=w[:], in_=wq_e[t, dk])
                wq_t[(t, dk)] = w
        mask128 = res_p.tile([128, 128], BF16)
        nc.gpsimd.dma_start(out=mask128[:], in_=mask_e[:])

        xkvs = {0: [], 1: []}

        def emit_xkv_g(b, g):
            t = xkv_p.tile([128, 4, S], BF16, tag="xkv", name=f"x{b}_{g}")
            for j in range(4):
                nc.sync.dma_start(out=t[:, j, :], in_=xkv_e[b, g][:, j, :])
            xkvs[b].append(t)

        def emit_xkv_load(b):
            for g in range(4):
                emit_xkv_g(b, g)

        def xk(b, dk):
            return xkvs[b][dk // 4][:, dk % 4, :]

        emit_xkv_load(0)

        kT = [res_p.tile([128, S], BF16, tag=f"kT{b}", name=f"kT{b}") for b in range(2)]
        qT = [res_p.tile([128, S], BF16, tag=f"qT{b}_{t}", name=f"qT{b}_{t}")
              for b in range(2) for t in range(2)]
        vplus = [[None] * 16, [None] * 16]

        buf_in_a = dram_p.tile([8, 128, 512], BF16)
        buf_in_b = dram_p.tile([8, 128, 512], BF16)
        buf_out_a = dram_p.tile([8, 128, 512], BF16)
        buf_out_b = dram_p.tile([8, 128, 512], BF16)

        def rope(dst_ap, ps_ap, sl, prange):
            plo, phi = prange
            raw = scr_p.tile([128, 512], F32, tag="raw", name="raw")
            nc.vector.tensor_copy(raw[plo:phi, :], ps_ap)
            pp = psOP.tile([128, 512], F32, tag="op", name="pp")
            nc.tensor.matmul(pp[plo:phi, :], permt[plo:phi, plo:phi], raw[plo:phi, :],
                             start=True, stop=True)
            nc.vector.tensor_mul(raw[plo:phi, :], raw[plo:phi, :], ropec[plo:phi, sl])
            t2 = scr_p.tile([128, 512], F32, tag="t2", name="t2")
            nc.vector.tensor_mul(t2[plo:phi, :], pp[plo:phi, :], ropeg[plo:phi, sl])
            nc.vector.tensor_add(dst_ap, raw[plo:phi, :], t2[plo:phi, :])

        def emit_K(b):
            # dk-outer over two [128,1024] slots (rows 0:64 used)
            pss = [psQ.tile([128, 1024], F32, tag="qk", name=f"kps{b}_{h}")
                   for h in range(2)]
            for dk in range(16):
                for sc in range(4):
                    nc.tensor.matmul(pss[sc // 2][0:64, (sc % 2) * 512:(sc % 2) * 512 + 512],
                                     wk_t[dk][:],
                                     xk(b, dk)[:, sc * 512:(sc + 1) * 512],
                                     start=(dk == 0), stop=(dk == 15))
            for sc in range(4):
                sl = slice(sc * 512, (sc + 1) * 512)
                rope(kT[b][0:64, sl],
                     pss[sc // 2][0:64, (sc % 2) * 512:(sc % 2) * 512 + 512], sl, (0, 64))
            nc.sync.dma_start(out=kT[b][64:128, :], in_=kT[b][0:64, :])

        def emit_V(b):
            for sc in range(16):
                ps = psQ.tile([128, 1024], F32, tag="qk", name=f"vps{b}_{sc}")
                for dk in range(16):
                    nc.tensor.matmul(ps[:, 0:64],
                                     xk(b, dk)[:, sc * 128:(sc + 1) * 128],
                                     wv_t[dk][:], start=(dk == 0), stop=(dk == 15))
                vt = vpl_p.tile([128, 65], BF16, tag="vplus", name=f"v{b}_{sc}")
                nc.vector.tensor_copy(vt[:, 0:64], ps[:, 0:64])
                nc.vector.memset(vt[:, 64:65], 1.0)
                vplus[b][sc] = vt

        def emit_Q_chunk(b, t, qc):
            ps = psPV.tile([128, 512], F32, tag="po", name=f"qps{b}_{t}_{qc}")
            for dk in range(16):
                nc.tensor.matmul(ps[:], wq_t[(t, dk)][:],
                                 xk(b, dk)[:, qc * 512:(qc + 1) * 512],
                                 start=(dk == 0), stop=(dk == 15))
            sl = slice(qc * 512, (qc + 1) * 512)
            rope(qT[b * 2 + t][:, sl], ps[:], sl, (0, 128))

        def attn_pair(t, b, qj):
            """Both heads of qT[b*2+t] for query block qj (512 queries)."""
            qt = qT[b * 2 + t]
            qs = qj * 512
            nk = 4 * (qj + 1)
            poA = psPV.tile([65, 512], F32, tag="po", name="poA")
            poB = psPV.tile([65, 512], F32, tag="po", name="poB")
            for r0 in range(0, nk, 4):
                pts = {}
                for kc in range(r0, r0 + 4):
                    mi = kc - 4 * qj
                    lo = mi * 128 if mi >= 0 else 0
                    psq = psQ.tile([128, 1024], F32, tag="qk", name="psq")
                    ksl = slice(kc * 128, (kc + 1) * 128)
                    nc.tensor.matmul(psq[:, lo:512], kT[b][0:64, ksl],
                                     qt[0:64, qs + lo:qs + 512],
                                     start=True, stop=True)
                    nc.tensor.matmul(psq[:, 512 + lo:1024], kT[b][64:128, ksl],
                                     qt[64:128, qs + lo:qs + 512],
                                     start=True, stop=True)
                    if mi >= 0:
                        nc.vector.tensor_add(psq[:, lo:lo + 128],
                                             psq[:, lo:lo + 128], mask128[:])
                        nc.vector.tensor_add(psq[:, 512 + lo:512 + lo + 128],
                                             psq[:, 512 + lo:512 + lo + 128], mask128[:])
                    pt = pt_p.tile([128, 1024], BF16, tag="pt", name="pt")
                    nc.scalar.activation(pt[:, 0:512], psq[:, 0:512],
                                         mybir.ActivationFunctionType.Exp, scale=0.125)
                    nc.scalar.activation(pt[:, 512:1024], psq[:, 512:1024],
                                         mybir.ActivationFunctionType.Exp, scale=0.125)
                    if lo > 0:
                        nc.vector.memset(pt[:, 0:lo], 0.0)
                        nc.vector.memset(pt[:, 512:512 + lo], 0.0)
                    pts[kc] = pt
                for kc in range(r0, r0 + 4):
                    nc.tensor.matmul(poA[:], vplus[b][kc][:], pts[kc][:, 0:512],
                                     start=(kc == 0), stop=(kc == nk - 1))
                    nc.tensor.matmul(poB[:], vplus[b][kc][:], pts[kc][:, 512:1024],
                                     start=(kc == 0), stop=(kc == nk - 1))
            # normalize: reciprocal of the ones-row, DRAM hop for the
            # partition-broadcast, then scale + emit to the A2A buffer.
            bi = buf_in_a if t == 0 else buf_in_b
            for hh, po in ((0, poA), (1, poB)):
                rc = nrm_p.tile([65, 512], F32, tag="rc", name="rc")
                nc.vector.reciprocal(rc[64:65, :], po[64:65, :])
                dr = dram_p.tile([1, 512], F32, tag="dnd", bufs=4, name="dr")
                nc.sync.dma_start(out=dr[:], in_=rc[64:65, :])
                dn = nrm_p.tile([64, 512], F32, tag="dn", name="dn")
                fl = dr[0]
                nc.sync.dma_start(out=dn[:],
                                  in_=bass.AP(tensor=fl.tensor, offset=fl.offset,
                                              ap=[[0, 64]] + list(fl.ap)))
                av = nrm_p.tile([64, 512], BF16, tag="av", name="av")
                nc.vector.tensor_mul(av[:], po[0:64, :], dn[:])
                nc.sync.dma_start(out=bi[b * 4 + qj, hh * 64:hh * 64 + 64, :], in_=av[:])

        # ---- output projection ----
        attn_full = {}
        ec_even = list(range(0, 16, 2))
        ec_odd = list(range(1, 16, 2))

        def emit_attn_full(ecs):
            for ec in ecs:
                t = pt_p.tile([128, 512], BF16, tag="af", bufs=16, name=f"af{ec}")
                bo = buf_out_a if ec % 2 == 0 else buf_out_b
                nc.gpsimd.dma_start(out=t[:], in_=bo[ec // 2, :, :])
                attn_full[ec] = t

        def emit_outproj(dcg, ecs, first):
            wts = {}
            for ec in ecs:
                wt = wst_p.tile([128, 512], BF16, tag="wo", name="wt")
                nc.sync.dma_start(out=wt[:], in_=wo_e[dcg, ec])
                wts[ec] = wt
            for j in range(4):           # sequential accumulator chains (1 bank)
                pss_out = psOP.tile([128, 512], F32, tag="op", name=f"op{dcg}_{j}")
                for i, ec in enumerate(ecs):
                    nc.tensor.matmul(pss_out[:],
                                     wts[ec][:, j * 128:(j + 1) * 128],
                                     attn_full[ec][:],
                                     start=(i == 0), stop=(i == len(ecs) - 1))
                dc = dcg * 4 + j
                ov = oev_p.tile([128, 512], F32, tag="oev", name="ov")
                nc.vector.tensor_copy(ov[:], pss_out[:])
                if first:
                    nc.sync.dma_start(out=out_e[dc * 128:(dc + 1) * 128, :], in_=ov[:])
                else:
                    nc.gpsimd.dma_start(out=out_e[dc * 128:(dc + 1) * 128, :], in_=ov[:],
                                        accum_op=mybir.AluOpType.add)

        # ================= emission schedule =================
        # phase A: batch-0 projections (K pipelines with the x DMA)
        emit_K(0)
        emit_V(0)
        for qc in range(4):
            emit_Q_chunk(0, 0, qc)

        def interleave(groups, steps):
            si = 0
            n = len(groups)
            for gi, g in enumerate(groups):
                attn_pair(*g)
                take = (len(steps) - si) // (n - gi)
                for _ in range(max(0, take)):
                    steps[si]()
                    si += 1
            while si < len(steps):
                steps[si]()
                si += 1

        # phase B: attn(t0, b0) x [Q(b0,t1), xkv1, K(b1), V(b1), Q(b1,t0)]
        stepsB = []
        stepsB.append(lambda: emit_xkv_g(1, 0))
        stepsB.append(lambda: emit_Q_chunk(0, 1, 0))
        stepsB.append(lambda: emit_xkv_g(1, 1))
        stepsB.append(lambda: emit_Q_chunk(0, 1, 1))
        stepsB.append(lambda: emit_xkv_g(1, 2))
        stepsB.append(lambda: emit_Q_chunk(0, 1, 2))
        stepsB.append(lambda: emit_xkv_g(1, 3))
        stepsB.append(lambda: emit_Q_chunk(0, 1, 3))
        stepsB.append(lambda: emit_K(1))
        stepsB.append(lambda: emit_V(1))
        for qc in range(4):
            stepsB.append(lambda qc=qc: emit_Q_chunk(1, 0, qc))
        interleave([(0, 0, qj) for qj in range(4)], stepsB)

        # phase C: attn(t0, b1) x [Q(b1,t1)]
        stepsC = [lambda qc=qc: emit_Q_chunk(1, 1, qc) for qc in range(4)]
        interleave([(0, 1, qj) for qj in range(4)], stepsC)

        nc.gpsimd.collective_compute(
            "AllToAll", mybir.AluOpType.bypass,
            ins=[buf_in_a.opt()], outs=[buf_out_a.opt()],
            replica_groups=[[0, 1, 2, 3, 4, 5, 6, 7]],
        )

        # phase D: attn(t1, b0) x [attn_full evens, outproj evens dcg 0-1]
        stepsD = [lambda: emit_attn_full(ec_even),
                  lambda: emit_outproj(0, ec_even, True),
                  lambda: emit_outproj(1, ec_even, True)]
        interleave([(1, 0, qj) for qj in range(4)], stepsD)

        # phase E: attn(t1, b1) x [outproj evens dcg 2]
        stepsE = [lambda: emit_outproj(2, ec_even, True)]
        interleave([(1, 1, qj) for qj in range(4)], stepsE)

        nc.gpsimd.collective_compute(
            "AllToAll", mybir.AluOpType.bypass,
            ins=[buf_in_b.opt()], outs=[buf_out_b.opt()],
            replica_groups=[[0, 1, 2, 3, 4, 5, 6, 7]],
        )

        # tail: dcg3 evens fill the A2A#2 wait; then the odd half accumulates
        emit_outproj(3, ec_even, True)
        emit_attn_full(ec_odd)
        for dcg in range(4):
            emit_outproj(dcg, ec_odd, False)

    nc.compile()
    return nc


def kernel(x, freqs_cos, freqs_sin, wq, wk, wv, wo, attn_mask):
    x = np.asarray(x, dtype=np.float32)
    freqs_cos = np.asarray(freqs_cos, dtype=np.float32)
    freqs_sin = np.asarray(freqs_sin, dtype=np.float32)
    wq = np.asarray(wq, dtype=np.float32)
    wk = np.asarray(wk, dtype=np.float32)
    wv = np.asarray(wv, dtype=np.float32)
    wo = np.asarray(wo, dtype=np.float32)
    attn_mask = np.asarray(attn_mask)
    assert np.all(attn_mask == 1), "kernel specialized for all-ones attn_mask"

    if "nc" not in _CACHE:
        _CACHE["nc"] = _build()
    nc = _CACHE["nc"]

    idx = np.arange(128)
    i_of_p = (idx % 64) // 2
    ropec = np.ascontiguousarray(freqs_cos.T[i_of_p].astype(np.float32))
    sgn = np.where(idx % 2 == 1, 1.0, -1.0).astype(np.float32)
    ropeg = np.ascontiguousarray((freqs_sin.T[i_of_p] * sgn[:, None]).astype(np.float32))
    perm = np.zeros((128, 128), np.float32)
    perm[idx, idx ^ 1] = 1.0
    m128 = np.where(np.arange(128)[:, None] > np.arange(128)[None, :],
                    np.float32(BF16_MIN), np.float32(0.0)).astype(BD)

    woT = np.ascontiguousarray(wo.T.astype(BD))
    wot = np.ascontiguousarray(woT.reshape(16, 128, 4, 512).transpose(2, 0, 1, 3))
    # xkv: [b, g, p, j, s] with x[b].T row (g*4+j)*128+p
    xT = x.transpose(0, 2, 1).astype(BD)                        # [b, d, s]
    xkv = np.ascontiguousarray(
        xT.reshape(2, 4, 4, 128, S).transpose(0, 1, 3, 2, 4))   # [b, g, p, j, s]

    in_maps = []
    for c in range(NCORES):
        wqr = wq[256 * c:256 * (c + 1)]
        wqt = np.ascontiguousarray(
            wqr.T.astype(BD).reshape(16, 128, 2, 128).transpose(2, 0, 1, 3))
        wkt = np.ascontiguousarray(wk[64 * c:64 * (c + 1)].T.astype(BD).reshape(16, 128, 64))
        wvt = np.ascontiguousarray(wv[64 * c:64 * (c + 1)].T.astype(BD).reshape(16, 128, 64))
        in_maps.append({
            "xkv": xkv, "wqt": wqt, "wkt": wkt, "wvt": wvt, "wot": wot,
            "ropec": ropec, "ropeg": ropeg, "perm": perm, "mask128": m128,
        })

    res = run_bass_kernel_spmd(nc, in_maps, core_ids=list(range(NCORES)))
    _CACHE["last_res"] = res

    out = np.empty((B, S, D), np.float32)
    for c in range(NCORES):
        b, r = c // 4, c % 4
        out[b, 512 * r:512 * (r + 1), :] = res.results[c]["out"].T
    return out


# revision 21
# speedup vs baseline: 1.2173x; 1.2173x over previous
"""Distributed GQA attention (B=2, S=2048, D=2048, H=32, KVH=8, HD=64,
causal + interleaved RoPE) on 8 Trainium2 NeuronCores.

Sharding (uniform SPMD -- one program, zero divergent control flow):
  Core c owns q-heads [4c, 4c+4) == exactly kv-head c, for BOTH batches.
  Two 8-core AllToAlls (bf16, 1MB each) re-shard the attention output from
  head-split to seq-split; after the A2As each core holds attn^T[all 2048
  features, its 512 q rows] and emits the FINAL out^T slice.

v2 structure (from perfetto/NTFF analysis of the 687us baseline):
  - QK for the two heads of a qT tile are emitted back-to-back with
    row-disjoint tile positions (rows 0:64 / 64:128) so the PE runs them
    CONCURRENTLY -- the attention QK stream halves.
  - exp runs as one [128,1024] ACTIVATE per kc (2 PSUM banks) instead of
    two [128,512] -- halves the 352-cycle per-instruction ACT overhead.
  - causal mask adds shrink from [128,512] to the [128,128] triangle
    block; fully-masked columns are skipped in QK and zeroed in pt.
  - softmax normalize: reciprocal on [1,512] rows directly, one DRAM hop
    for the partition-broadcast (was 3).
  - scalar queue carries ONLY activations (DMAs moved to sync/gpsimd) --
    the baseline's 140us of scalar-queue DMA starved exp and stalled the
    PE on PSUM WAR at the A2A windows.
  - x loads split into 512KB (g,j) chunks so projections chase the DMA.
"""
import sys
if '/opt/trn_rl_repo' not in sys.path:
    sys.path.insert(0, '/opt/trn_rl_repo')

import numpy as np
import ml_dtypes
from contextlib import ExitStack

import concourse.bass as bass
import concourse.bacc as bacc
import concourse.tile as tile
from concourse import mybir
from concourse.bass_utils import run_bass_kernel_spmd

B, S, D = 2, 2048, 2048
H, KVH, HD = 32, 8, 64
NCORES = 8
BF16_MIN = -3.3895313892515355e+38
BF16 = mybir.dt.bfloat16
F32 = mybir.dt.float32
BD = ml_dtypes.bfloat16

_CACHE = {}


def _build():
    nc = bacc.Bacc("TRN2", target_bir_lowering=False, debug=False,
                   num_devices=NCORES, name="attn")

    xkv_e = nc.declare_dram_parameter("xkv", [2, 4, 128, 4, S], BF16, False)   # [b, g, p, j, s]
    wq_e = nc.declare_dram_parameter("wqt", [2, 16, 128, 128], BF16, False)    # [t, dk, d, e]
    wk_e = nc.declare_dram_parameter("wkt", [16, 128, 64], BF16, False)
    wv_e = nc.declare_dram_parameter("wvt", [16, 128, 64], BF16, False)
    wo_e = nc.declare_dram_parameter("wot", [4, 16, 128, 512], BF16, False)    # [dcg, ec, e, d]
    ropec_e = nc.declare_dram_parameter("ropec", [128, S], F32, False)
    ropeg_e = nc.declare_dram_parameter("ropeg", [128, S], F32, False)
    perm_e = nc.declare_dram_parameter("perm", [128, 128], F32, False)
    mask_e = nc.declare_dram_parameter("mask128", [128, 128], BF16, False)
    out_e = nc.declare_dram_parameter("out", [D, 512], F32, True)

    with tile.TileContext(nc) as tc, ExitStack() as ctx:
        xkv_p = ctx.enter_context(tc.tile_pool(name="xkv", bufs=4))
        res_p = ctx.enter_context(tc.tile_pool(name="res", bufs=1))
        scr_p = ctx.enter_context(tc.tile_pool(name="scr", bufs=2))
        pt_p = ctx.enter_context(tc.tile_pool(name="pt", bufs=5))
        nrm_p = ctx.enter_context(tc.tile_pool(name="nrm", bufs=4))
        oev_p = ctx.enter_context(tc.tile_pool(name="oev", bufs=4))
        wst_p = ctx.enter_context(tc.tile_pool(name="wst", bufs=20))
        vpl_p = ctx.enter_context(tc.tile_pool(name="vpl", bufs=32))
        dram_p = ctx.enter_context(tc.tile_pool(name="dram", bufs=1, space="DRAM"))
        # PSUM: 8 banks total.  qk quads [128,1024]x2 = 4, po [65,512]x3 = 3,
        # op (outproj / rope-pp) [128,512]x1 = 1.
        psQ = ctx.enter_context(tc.tile_pool(name="psQ", bufs=2, space="PSUM"))
        psPV = ctx.enter_context(tc.tile_pool(name="psPV", bufs=3, space="PSUM"))
        psOP = ctx.enter_context(tc.tile_pool(name="psOP", bufs=1, space="PSUM"))

        # ---- resident constants: weights on gpsimd queue, x on sync ----
        wk_t, wv_t, wq_t = [], [], {}
        for dk in range(16):
            a = res_p.tile([128, 64], BF16, tag=f"wk{dk}", name=f"wk{dk}")
            nc.gpsimd.dma_start(out=a[:], in_=wk_e[dk])
            wk_t.append(a)
        ropec = res_p.tile([128, S], F32)
        ropeg = res_p.tile([128, S], F32)
        permt = res_p.tile([128, 128], F32)
        nc.gpsimd.dma_start(out=ropec[:], in_=ropec_e[:])
        nc.gpsimd.dma_start(out=ropeg[:], in_=ropeg_e[:])
        nc.gpsimd.dma_start(out=permt[:], in_=perm_e[:])
        for dk in range(16):
            bb = res_p.tile([128, 64], BF16, tag=f"wv{dk}", name=f"wv{dk}")
            nc.gpsimd.dma_start(out=bb[:], in_=wv_e[dk])
            wv_t.append(bb)
        for t in range(2):
            for dk in range(16):
                w = res_p.tile([128, 128], BF16, tag=f"wq{t}_{dk}", name=f"wq{t}_{dk}")
                nc.gpsimd.dma_start(out=w[:], in_=wq_e[t, dk])
                wq_t[(t, dk)] = w
        mask128 = res_p.tile([128, 128], BF16)
        nc.gpsimd.dma_start(out=mask128[:], in_=mask_e[:])

        xkvs = {0: [], 1: []}

        def emit_xkv_g(b, g, eng=None):
            t = xkv_p.tile([128, 4, S], BF16, tag="xkv", name=f"x{b}_{g}")
            for j in range(4):
                e = eng if eng is not None else nc.sync
                e.dma_start(out=t[:, j, :], in_=xkv_e[b, g][:, j, :])
            xkvs[b].append(t)

        def emit_xkv_load(b):
            # split across the idle-at-startup sync + scalar queues
            for g in range(4):
                emit_xkv_g(b, g, eng=(nc.sync if g % 2 == 0 else nc.scalar))

        def xk(b, dk):
            return xkvs[b][dk // 4][:, dk % 4, :]

        emit_xkv_load(0)

        kT = [res_p.tile([128, S], BF16, tag=f"kT{b}", name=f"kT{b}") for b in range(2)]
        qT = [res_p.tile([128, S], BF16, tag=f"qT{b}_{t}", name=f"qT{b}_{t}")
              for b in range(2) for t in range(2)]
        vplus = [[None] * 16, [None] * 16]

        buf_in_a = dram_p.tile([8, 128, 512], BF16)
        buf_in_b = dram_p.tile([8, 128, 512], BF16)
        buf_out_a = dram_p.tile([8, 128, 512], BF16)
        buf_out_b = dram_p.tile([8, 128, 512], BF16)

        def rope(dst_ap, ps_ap, sl, prange):
            plo, phi = prange
            raw = scr_p.tile([128, 512], F32, tag="raw", name="raw")
            nc.vector.tensor_copy(raw[plo:phi, :], ps_ap)
            pp = psOP.tile([128, 512], F32, tag="op", name="pp")
            nc.tensor.matmul(pp[plo:phi, :], permt[plo:phi, plo:phi], raw[plo:phi, :],
                             start=True, stop=True)
            nc.vector.tensor_mul(raw[plo:phi, :], raw[plo:phi, :], ropec[plo:phi, sl])
            t2 = scr_p.tile([128, 512], F32, tag="t2", name="t2")
            nc.vector.tensor_mul(t2[plo:phi, :], pp[plo:phi, :], ropeg[plo:phi, sl])
            nc.vector.tensor_add(dst_ap, raw[plo:phi, :], t2[plo:phi, :])

        def emit_K(b):
            # dk-outer; sc pairs stacked on out partitions (col-tiled) so the
            # two 64-col K matmuls run concurrently in disjoint col groups.
            # start=True clears has_written for the WHOLE bank, so only the
            # chronologically-first matmul per bank may carry it.
            pss = psQ.tile([128, 1024], F32, tag="qk", name=f"kps{b}")
            for dk in range(16):
                for sc in range(4):
                    rlo = (sc % 2) * 64
                    clo = (sc // 2) * 512
                    nc.tensor.matmul(pss[rlo:rlo + 64, clo:clo + 512],
                                     wk_t[dk][:],
                                     xk(b, dk)[:, sc * 512:(sc + 1) * 512],
                                     start=(dk == 0), stop=(dk == 15),
                                     tile_position=(0, rlo), skip_group_check=True)
            for sc in range(4):
                rlo = (sc % 2) * 64
                clo = (sc // 2) * 512
                sl = slice(sc * 512, (sc + 1) * 512)
                # odd sc rope'd in rows 64:128 (tables are 64-periodic), lands
                # directly in the kT duplicate rows; dup DMAs fill the rest.
                rope(kT[b][rlo:rlo + 64, sl], pss[rlo:rlo + 64, clo:clo + 512],
                     sl, (rlo, rlo + 64))
            nc.sync.dma_start(out=kT[b][64:128, 0:512], in_=kT[b][0:64, 0:512])
            nc.sync.dma_start(out=kT[b][0:64, 512:1024], in_=kT[b][64:128, 512:1024])
            nc.sync.dma_start(out=kT[b][64:128, 1024:1536], in_=kT[b][0:64, 1024:1536])
            nc.sync.dma_start(out=kT[b][0:64, 1536:2048], in_=kT[b][64:128, 1536:2048])

        def emit_V(b):
            # dk-outer into one [128,1024] slot: 16 sc chunks of 64 cols.
            # One start per bank (sc==0 clears bank0, sc==8 clears bank1).
            pss = psQ.tile([128, 1024], F32, tag="qk", name=f"vps{b}")
            for dk in range(16):
                for sc in range(16):
                    nc.tensor.matmul(pss[:, sc * 64:sc * 64 + 64],
                                     xk(b, dk)[:, sc * 128:(sc + 1) * 128],
                                     wv_t[dk][:],
                                     start=(dk == 0 and sc % 8 == 0), stop=(dk == 15),
                                     skip_group_check=True)
            for sc in range(16):
                vt = vpl_p.tile([128, 65], BF16, tag="vplus", name=f"v{b}_{sc}")
                nc.vector.tensor_copy(vt[:, 0:64], pss[:, sc * 64:sc * 64 + 64])
                nc.vector.memset(vt[:, 64:65], 1.0)
                vplus[b][sc] = vt

        def emit_Q_chunk(b, t, qc):
            ps = psPV.tile([128, 512], F32, tag="po", name=f"qps{b}_{t}_{qc}")
            for dk in range(16):
                nc.tensor.matmul(ps[:], wq_t[(t, dk)][:],
                                 xk(b, dk)[:, qc * 512:(qc + 1) * 512],
                                 start=(dk == 0), stop=(dk == 15))
            sl = slice(qc * 512, (qc + 1) * 512)
            rope(qT[b * 2 + t][:, sl], ps[:], sl, (0, 128))

        def attn_pair(t, b, qj):
            """Both heads of qT[b*2+t] for query block qj (512 queries)."""
            qt = qT[b * 2 + t]
            qs = qj * 512
            nk = 4 * (qj + 1)
            poA = psPV.tile([65, 512], F32, tag="po", name="poA")
            poB = psPV.tile([65, 512], F32, tag="po", name="poB")
            for r0 in range(0, nk, 4):
                pts = {}
                for kc in range(r0, r0 + 4):
                    mi = kc - 4 * qj
                    lo = mi * 128 if mi >= 0 else 0
                    psq = psQ.tile([128, 1024], F32, tag="qk", name="psq")
                    ksl = slice(kc * 128, (kc + 1) * 128)
                    nc.tensor.matmul(psq[:, lo:512], kT[b][0:64, ksl],
                                     qt[0:64, qs + lo:qs + 512],
                                     start=True, stop=True)
                    nc.tensor.matmul(psq[:, 512 + lo:1024], kT[b][64:128, ksl],
                                     qt[64:128, qs + lo:qs + 512],
                                     start=True, stop=True)
                    if mi >= 0:
                        nc.vector.tensor_add(psq[:, lo:lo + 128],
                                             psq[:, lo:lo + 128], mask128[:])
                        nc.vector.tensor_add(psq[:, 512 + lo:512 + lo + 128],
                                             psq[:, 512 + lo:512 + lo + 128], mask128[:])
                    pt = pt_p.tile([128, 1024], BF16, tag="pt", name="pt")
                    nc.scalar.activation(pt[:], psq[:],
                                         mybir.ActivationFunctionType.Exp, scale=0.125)
                    if lo > 0:
                        nc.vector.memset(pt[:, 0:lo], 0.0)
                        nc.vector.memset(pt[:, 512:512 + lo], 0.0)
                    pts[kc] = pt
                for kc in range(r0, r0 + 4):
                    nc.tensor.matmul(poA[:], vplus[b][kc][:], pts[kc][:, 0:512],
                                     start=(kc == 0), stop=(kc == nk - 1))
                    nc.tensor.matmul(poB[:], vplus[b][kc][:], pts[kc][:, 512:1024],
                                     start=(kc == 0), stop=(kc == nk - 1))
            # normalize: reciprocal of the ones-row, DRAM hop for the
            # partition-broadcast, then scale + emit to the A2A buffer.
            bi = buf_in_a if t == 0 else buf_in_b
            for hh, po in ((0, poA), (1, poB)):
                rc = nrm_p.tile([65, 512], F32, tag="rc", name="rc", bufs=2)
                nc.vector.reciprocal(rc[64:65, :], po[64:65, :])
                dr = dram_p.tile([1, 512], F32, tag="dnd", bufs=4, name="dr")
                nc.sync.dma_start(out=dr[:], in_=rc[64:65, :])
                dn = nrm_p.tile([64, 512], F32, tag="dn", name="dn", bufs=2)
                fl = dr[0]
                nc.sync.dma_start(out=dn[:],
                                  in_=bass.AP(tensor=fl.tensor, offset=fl.offset,
                                              ap=[[0, 64]] + list(fl.ap)))
                av = nrm_p.tile([64, 512], BF16, tag="av", name="av")
                nc.vector.tensor_mul(av[:], po[0:64, :], dn[:])
                nc.gpsimd.dma_start(out=bi[b * 4 + qj, hh * 64:hh * 64 + 64, :], in_=av[:])

        # ---- output projection ----
        attn_full = {}
        ec_even = list(range(0, 16, 2))
        ec_odd = list(range(1, 16, 2))

        def emit_attn_full(ecs):
            for ec in ecs:
                t = pt_p.tile([128, 512], BF16, tag="af", bufs=16, name=f"af{ec}")
                bo = buf_out_a if ec % 2 == 0 else buf_out_b
                nc.gpsimd.dma_start(out=t[:], in_=bo[ec // 2, :, :])
                attn_full[ec] = t

        wo_tiles = {}

        def load_wo(dcg, ecs):
            for ec in ecs:
                wt = wst_p.tile([128, 512], BF16, tag="wo", name="wt")
                nc.sync.dma_start(out=wt[:], in_=wo_e[dcg, ec])
                wo_tiles[(dcg, ec)] = wt

        def emit_outproj(dcg, ecs, first):
            for j in range(4):           # sequential accumulator chains (1 bank)
                pss_out = psOP.tile([128, 512], F32, tag="op", name=f"op{dcg}_{j}")
                for i, ec in enumerate(ecs):
                    nc.tensor.matmul(pss_out[:],
                                     wo_tiles[(dcg, ec)][:, j * 128:(j + 1) * 128],
                                     attn_full[ec][:],
                                     start=(i == 0), stop=(i == len(ecs) - 1))
                dc = dcg * 4 + j
                ov = oev_p.tile([128, 512], F32, tag="oev", name="ov")
                nc.vector.tensor_copy(ov[:], pss_out[:])
                if first:
                    nc.sync.dma_start(out=out_e[dc * 128:(dc + 1) * 128, :], in_=ov[:])
                else:
                    nc.gpsimd.dma_start(out=out_e[dc * 128:(dc + 1) * 128, :], in_=ov[:],
                                        accum_op=mybir.AluOpType.add)

        # ================= emission schedule =================
        # phase A: batch-0 projections (K pipelines with the x DMA)
        emit_K(0)
        emit_V(0)
        for qc in range(4):
            emit_Q_chunk(0, 0, qc)

        def interleave(groups, steps):
            si = 0
            n = len(groups)
            for gi, g in enumerate(groups):
                attn_pair(*g)
                take = (len(steps) - si) // (n - gi)
                for _ in range(max(0, take)):
                    steps[si]()
                    si += 1
            while si < len(steps):
                steps[si]()
                si += 1

        # phase B: attn(t0, b0) x [Q(b0,t1), xkv1, K(b1), V(b1), Q(b1,t0)]
        stepsB = []
        for qc in range(4):
            stepsB.append(lambda qc=qc: emit_Q_chunk(0, 1, qc))
        for g in range(4):
            stepsB.append(lambda g=g: emit_xkv_g(1, g))
        stepsB.append(lambda: emit_K(1))
        stepsB.append(lambda: emit_V(1))
        for qc in range(4):
            stepsB.append(lambda qc=qc: emit_Q_chunk(1, 0, qc))
        interleave([(0, 0, qj) for qj in range(4)], stepsB)

        # phase C: attn(t0, b1) x [Q(b1,t1)]
        stepsC = [lambda qc=qc: emit_Q_chunk(1, 1, qc) for qc in range(4)]
        interleave([(0, 1, qj) for qj in range(4)], stepsC)

        nc.gpsimd.collective_compute(
            "AllToAll", mybir.AluOpType.bypass,
            ins=[buf_in_a.opt()], outs=[buf_out_a.opt()],
            replica_groups=[[0, 1, 2, 3, 4, 5, 6, 7]],
        )

        # phase D: attn(t1, b0) x [attn_full evens, outproj evens dcg 0-1]
        stepsD = [lambda: (emit_attn_full(ec_even), load_wo(0, ec_even),
                           load_wo(1, ec_even)),
                  lambda: emit_outproj(0, ec_even, True),
                  lambda: emit_outproj(1, ec_even, True)]
        interleave([(1, 0, qj) for qj in range(4)], stepsD)

        # phase E: attn(t1, b1) x [outproj evens dcg 2]; dcg2/3 wo loads land on
        # sync BEFORE outproj(2)'s out-writes so the A2A#2-window work (dcg3)
        # is never head-of-line blocked.
        stepsE = [lambda: (load_wo(2, ec_even), load_wo(3, ec_even)),
                  lambda: emit_outproj(2, ec_even, True)]
        interleave([(1, 1, qj) for qj in range(4)], stepsE)

        nc.gpsimd.collective_compute(
            "AllToAll", mybir.AluOpType.bypass,
            ins=[buf_in_b.opt()], outs=[buf_out_b.opt()],
            replica_groups=[[0, 1, 2, 3, 4, 5, 6, 7]],
        )

        # tail: dcg3 evens fill the A2A#2 wait; then the odd half accumulates
        emit_outproj(3, ec_even, True)
        emit_attn_full(ec_odd)
        for dcg in range(4):
            load_wo(dcg, ec_odd)
            emit_outproj(dcg, ec_odd, False)

    nc.compile()
    return nc


def kernel(x, freqs_cos, freqs_sin, wq, wk, wv, wo, attn_mask):
    x = np.asarray(x, dtype=np.float32)
    freqs_cos = np.asarray(freqs_cos, dtype=np.float32)
    freqs_sin = np.asarray(freqs_sin, dtype=np.float32)
    wq = np.asarray(wq, dtype=np.float32)
    wk = np.asarray(wk, dtype=np.float32)
    wv = np.asarray(wv, dtype=np.float32)
    wo = np.asarray(wo, dtype=np.float32)
    attn_mask = np.asarray(attn_mask)
    assert np.all(attn_mask == 1), "kernel specialized for all-ones attn_mask"

    if "nc" not in _CACHE:
        _CACHE["nc"] = _build()
    nc = _CACHE["nc"]

    idx = np.arange(128)
    i_of_p = (idx % 64) // 2
    ropec = np.ascontiguousarray(freqs_cos.T[i_of_p].astype(np.float32))
    sgn = np.where(idx % 2 == 1, 1.0, -1.0).astype(np.float32)
    ropeg = np.ascontiguousarray((freqs_sin.T[i_of_p] * sgn[:, None]).astype(np.float32))
    perm = np.zeros((128, 128), np.float32)
    perm[idx, idx ^ 1] = 1.0
    m128 = np.where(np.arange(128)[:, None] > np.arange(128)[None, :],
                    np.float32(BF16_MIN), np.float32(0.0)).astype(BD)

    woT = np.ascontiguousarray(wo.T.astype(BD))
    wot = np.ascontiguousarray(woT.reshape(16, 128, 4, 512).transpose(2, 0, 1, 3))
    # xkv: [b, g, p, j, s] with x[b].T row (g*4+j)*128+p
    xT = x.transpose(0, 2, 1).astype(BD)                        # [b, d, s]
    xkv = np.ascontiguousarray(
        xT.reshape(2, 4, 4, 128, S).transpose(0, 1, 3, 2, 4))   # [b, g, p, j, s]

    in_maps = []
    for c in range(NCORES):
        wqr = wq[256 * c:256 * (c + 1)]
        wqt = np.ascontiguousarray(
            wqr.T.astype(BD).reshape(16, 128, 2, 128).transpose(2, 0, 1, 3))
        wkt = np.ascontiguousarray(wk[64 * c:64 * (c + 1)].T.astype(BD).reshape(16, 128, 64))
        wvt = np.ascontiguousarray(wv[64 * c:64 * (c + 1)].T.astype(BD).reshape(16, 128, 64))
        in_maps.append({
            "xkv": xkv, "wqt": wqt, "wkt": wkt, "wvt": wvt, "wot": wot,
            "ropec": ropec, "ropeg": ropeg, "perm": perm, "mask128": m128,
        })

    res = run_bass_kernel_spmd(nc, in_maps, core_ids=list(range(NCORES)))
    _CACHE["last_res"] = res

    out = np.empty((B, S, D), np.float32)
    for c in range(NCORES):
        b, r = c // 4, c % 4
        out[b, 512 * r:512 * (r + 1), :] = res.results[c]["out"].T
    return out


# revision 32
# speedup vs baseline: 1.4530x; 1.1936x over previous
"""Distributed GQA attention (B=2, S=2048, D=2048, H=32, KVH=8, HD=64,
causal + interleaved RoPE) on 8 Trainium2 NeuronCores.

Sharding (uniform SPMD -- one program, zero divergent control flow):
  Core c owns q-heads [4c, 4c+4) == exactly kv-head c, for BOTH batches.
  Two 8-core AllToAlls (bf16, 1MB each) re-shard the attention output from
  head-split to seq-split; after the A2As each core holds attn^T[all 2048
  features, its 512 q rows] and emits the FINAL out^T slice.

Structure (from perfetto/NTFF analysis of the 687us baseline; now ~575us):
  - QK for the two heads of a qT tile are emitted back-to-back with
    row-disjoint tile positions (rows 0:64 / 64:128) so the PE runs them
    CONCURRENTLY -- the attention QK stream halves.  K-proj sc pairs are
    col-tiled the same way (out partitions 0:64 / 64:128).
  - exp runs as one [128,1024] ACTIVATE per kc (a 2-bank PSUM quad holding
    both heads) -- halves the 352-cycle per-ACT overhead vs [128,512]x2.
  - causal mask adds shrink from [128,512] to the [128,128] triangle
    block; fully-masked columns are skipped in QK and zeroed in pt.
  - PSUM zero-region rule (hard-won): start=True marks the whole 2KB
    region pending-zero FOR THE PARTITIONS THE MATMUL WRITES.  Multiple
    interleaved accumulation groups may share a bank only if they cover
    disjoint partition ranges (each starts its own rows), or if exactly
    the first group's start covers all 128 partitions (V proj: one start
    per bank at sc 0/8).  Violations silently drop dk-0 contributions or
    accumulate onto stale +-3e38 garbage (NaN).
  - softmax normalize: DVE reciprocal on the [1,512] ones-row directly,
    one DRAM hop for the partition-broadcast (was 3).
  - scalar queue carries only ACTs (+ startup x chunks); normalize DMAs
    on sync, NOT behind the blocking collective wait on gpsimd.
  - outproj evens for dcg 1-3 are deferred past the A2A#2 emission to
    fill the collective window; tail chains alternate psOP/psQ banks so
    chain j+1's matmuls overlap chain j's PSUM copy-out.
  - x loads split into 512KB (g,j) chunks across sync+scalar queues so
    the dk-outer projections chase the DMA from ~2us in.
"""
import sys
if '/opt/trn_rl_repo' not in sys.path:
    sys.path.insert(0, '/opt/trn_rl_repo')

import numpy as np
import ml_dtypes
from contextlib import ExitStack

import concourse.bass as bass
import concourse.bacc as bacc
import concourse.tile as tile
from concourse import mybir
from concourse.bass_utils import run_bass_kernel_spmd

B, S, D = 2, 2048, 2048
H, KVH, HD = 32, 8, 64
NCORES = 8
BF16_MIN = -3.3895313892515355e+38
BF16 = mybir.dt.bfloat16
F32 = mybir.dt.float32
BD = ml_dtypes.bfloat16

_CACHE = {}


def _build():
    nc = bacc.Bacc("TRN2", target_bir_lowering=False, debug=False,
                   num_devices=NCORES, name="attn")

    xkv_e = nc.declare_dram_parameter("xkv", [2, 4, 128, 4, S], BF16, False)   # [b, g, p, j, s]
    wq_e = nc.declare_dram_parameter("wqt", [2, 16, 128, 128], BF16, False)    # [t, dk, d, e]
    wk_e = nc.declare_dram_parameter("wkt", [16, 128, 64], BF16, False)
    wv_e = nc.declare_dram_parameter("wvt", [16, 128, 64], BF16, False)
    wo_e = nc.declare_dram_parameter("wot", [4, 16, 128, 512], BF16, False)    # [dcg, ec, e, d]
    ropec_e = nc.declare_dram_parameter("ropec", [128, S], F32, False)
    ropeg_e = nc.declare_dram_parameter("ropeg", [128, S], F32, False)
    perm_e = nc.declare_dram_parameter("perm", [128, 128], F32, False)
    mask_e = nc.declare_dram_parameter("mask128", [128, 128], BF16, False)
    out_e = nc.declare_dram_parameter("out", [D, 512], F32, True)

    with tile.TileContext(nc) as tc, ExitStack() as ctx:
        xkv_p = ctx.enter_context(tc.tile_pool(name="xkv", bufs=4))
        res_p = ctx.enter_context(tc.tile_pool(name="res", bufs=1))
        scr_p = ctx.enter_context(tc.tile_pool(name="scr", bufs=2))
        pt_p = ctx.enter_context(tc.tile_pool(name="pt", bufs=6))
        nrm_p = ctx.enter_context(tc.tile_pool(name="nrm", bufs=4))
        oev_p = ctx.enter_context(tc.tile_pool(name="oev", bufs=4))
        wst_p = ctx.enter_context(tc.tile_pool(name="wst", bufs=20))
        vpl_p = ctx.enter_context(tc.tile_pool(name="vpl", bufs=32))
        dram_p = ctx.enter_context(tc.tile_pool(name="dram", bufs=1, space="DRAM"))
        # PSUM: 8 banks total.  qk quads [128,1024]x2 = 4, po [65,512]x3 = 3,
        # op (outproj / rope-pp) [128,512]x1 = 1.
        psQ = ctx.enter_context(tc.tile_pool(name="psQ", bufs=2, space="PSUM"))
        psPV = ctx.enter_context(tc.tile_pool(name="psPV", bufs=3, space="PSUM"))
        psOP = ctx.enter_context(tc.tile_pool(name="psOP", bufs=1, space="PSUM"))

        # ---- resident constants: weights on gpsimd queue, x on sync ----
        wq_t = {}
        wkall = res_p.tile([128, 16, 64], BF16)
        nc.gpsimd.dma_start(out=wkall[:], in_=wk_e[:].rearrange("k p f -> p k f"))
        wk_t = [wkall[:, dk, :] for dk in range(16)]
        ropec = res_p.tile([128, S], F32)
        ropeg = res_p.tile([128, S], F32)
        permt = res_p.tile([128, 128], F32)
        nc.gpsimd.dma_start(out=ropec[:], in_=ropec_e[:])
        nc.gpsimd.dma_start(out=ropeg[:], in_=ropeg_e[:])
        nc.gpsimd.dma_start(out=permt[:], in_=perm_e[:])
        wvall = res_p.tile([128, 16, 64], BF16)
        nc.gpsimd.dma_start(out=wvall[:], in_=wv_e[:].rearrange("k p f -> p k f"))
        wv_t = [wvall[:, dk, :] for dk in range(16)]
        for t in range(2):
            for dk in range(16):
                w = res_p.tile([128, 128], BF16, tag=f"wq{t}_{dk}", name=f"wq{t}_{dk}")
                nc.gpsimd.dma_start(out=w[:], in_=wq_e[t, dk])
                wq_t[(t, dk)] = w
        mask128 = res_p.tile([128, 128], BF16)
        nc.gpsimd.dma_start(out=mask128[:], in_=mask_e[:])

        xkvs = {0: [], 1: []}

        def emit_xkv_g(b, g, eng=None):
            t = xkv_p.tile([128, 4, S], BF16, tag="xkv", name=f"x{b}_{g}")
            for j in range(4):
                e = eng if eng is not None else nc.sync
                e.dma_start(out=t[:, j, :], in_=xkv_e[b, g][:, j, :])
            xkvs[b].append(t)

        def emit_xkv_load(b):
            # split across the idle-at-startup sync + scalar queues
            for g in range(4):
                emit_xkv_g(b, g, eng=(nc.sync if g % 2 == 0 else nc.scalar))

        def xk(b, dk):
            return xkvs[b][dk // 4][:, dk % 4, :]

        emit_xkv_load(0)

        kT = [res_p.tile([128, S], BF16, tag=f"kT{b}", name=f"kT{b}") for b in range(2)]
        qT = [res_p.tile([128, S], BF16, tag=f"qT{b}_{t}", name=f"qT{b}_{t}")
              for b in range(2) for t in range(2)]
        vplus = [[None] * 16, [None] * 16]

        buf_in_a = dram_p.tile([8, 128, 512], BF16)
        buf_in_b = dram_p.tile([8, 128, 512], BF16)
        buf_out_a = dram_p.tile([8, 128, 512], BF16)
        buf_out_b = dram_p.tile([8, 128, 512], BF16)

        def rope(dst_ap, ps_ap, sl, prange):
            plo, phi = prange
            raw = scr_p.tile([128, 512], F32, tag="raw", name="raw")
            nc.vector.tensor_copy(raw[plo:phi, :], ps_ap)
            pp = psOP.tile([128, 512], F32, tag="op", name="pp")
            nc.tensor.matmul(pp[plo:phi, :], permt[plo:phi, plo:phi], raw[plo:phi, :],
                             start=True, stop=True)
            nc.vector.tensor_mul(raw[plo:phi, :], raw[plo:phi, :], ropec[plo:phi, sl])
            t2 = scr_p.tile([128, 512], F32, tag="t2", name="t2")
            nc.vector.tensor_mul(t2[plo:phi, :], pp[plo:phi, :], ropeg[plo:phi, sl])
            nc.vector.tensor_add(dst_ap, raw[plo:phi, :], t2[plo:phi, :])

        def emit_K(b):
            # dk-outer; sc pairs stacked on out partitions (col-tiled) so the
            # two 64-col K matmuls run concurrently in disjoint col groups.
            # start=True clears has_written for the WHOLE bank, so only the
            # chronologically-first matmul per bank may carry it.
            pss = psQ.tile([128, 1024], F32, tag="qk", name=f"kps{b}")
            for dk in range(16):
                for sc in range(4):
                    rlo = (sc % 2) * 64
                    clo = (sc // 2) * 512
                    nc.tensor.matmul(pss[rlo:rlo + 64, clo:clo + 512],
                                     wk_t[dk][:],
                                     xk(b, dk)[:, sc * 512:(sc + 1) * 512],
                                     start=(dk == 0), stop=(dk == 15),
                                     tile_position=(0, rlo), skip_group_check=True)
            for sc in range(4):
                rlo = (sc % 2) * 64
                clo = (sc // 2) * 512
                sl = slice(sc * 512, (sc + 1) * 512)
                # odd sc rope'd in rows 64:128 (tables are 64-periodic), lands
                # directly in the kT duplicate rows; dup DMAs fill the rest.
                rope(kT[b][rlo:rlo + 64, sl], pss[rlo:rlo + 64, clo:clo + 512],
                     sl, (rlo, rlo + 64))
            nc.sync.dma_start(out=kT[b][64:128, 0:512], in_=kT[b][0:64, 0:512])
            nc.sync.dma_start(out=kT[b][0:64, 512:1024], in_=kT[b][64:128, 512:1024])
            nc.sync.dma_start(out=kT[b][64:128, 1024:1536], in_=kT[b][0:64, 1024:1536])
            nc.sync.dma_start(out=kT[b][0:64, 1536:2048], in_=kT[b][64:128, 1536:2048])

        def emit_V(b):
            # dk-outer into one [128,1024] slot: 16 sc chunks of 64 cols.
            # One start per bank (sc==0 clears bank0, sc==8 clears bank1).
            pss = psQ.tile([128, 1024], F32, tag="qk", name=f"vps{b}")
            for dk in range(16):
                for sc in range(16):
                    nc.tensor.matmul(pss[:, sc * 64:sc * 64 + 64],
                                     xk(b, dk)[:, sc * 128:(sc + 1) * 128],
                                     wv_t[dk][:],
                                     start=(dk == 0 and sc % 8 == 0), stop=(dk == 15),
                                     skip_group_check=True)
            for sc in range(16):
                vt = vpl_p.tile([128, 65], BF16, tag="vplus", name=f"v{b}_{sc}")
                nc.vector.tensor_copy(vt[:, 0:64], pss[:, sc * 64:sc * 64 + 64])
                nc.vector.memset(vt[:, 64:65], 1.0)
                vplus[b][sc] = vt

        def emit_Q_chunk(b, t, qc):
            ps = psPV.tile([128, 512], F32, tag="po", name=f"qps{b}_{t}_{qc}")
            for dk in range(16):
                nc.tensor.matmul(ps[:], wq_t[(t, dk)][:],
                                 xk(b, dk)[:, qc * 512:(qc + 1) * 512],
                                 start=(dk == 0), stop=(dk == 15))
            sl = slice(qc * 512, (qc + 1) * 512)
            rope(qT[b * 2 + t][:, sl], ps[:], sl, (0, 128))

        def attn_pair(t, b, qj):
            """Both heads of qT[b*2+t] for query block qj (512 queries)."""
            qt = qT[b * 2 + t]
            qs = qj * 512
            nk = 4 * (qj + 1)
            poA = psPV.tile([65, 512], F32, tag="po", name="poA")
            poB = psPV.tile([65, 512], F32, tag="po", name="poB")
            for r0 in range(0, nk, 4):
                pts = {}
                for kc in range(r0, r0 + 4):
                    mi = kc - 4 * qj
                    lo = mi * 128 if mi >= 0 else 0
                    psq = psQ.tile([128, 1024], F32, tag="qk", name="psq")
                    ksl = slice(kc * 128, (kc + 1) * 128)
                    nc.tensor.matmul(psq[:, lo:512], kT[b][0:64, ksl],
                                     qt[0:64, qs + lo:qs + 512],
                                     start=True, stop=True)
                    nc.tensor.matmul(psq[:, 512 + lo:1024], kT[b][64:128, ksl],
                                     qt[64:128, qs + lo:qs + 512],
                                     start=True, stop=True)
                    if mi >= 0:
                        nc.vector.tensor_add(psq[:, lo:lo + 128],
                                             psq[:, lo:lo + 128], mask128[:])
                        nc.vector.tensor_add(psq[:, 512 + lo:512 + lo + 128],
                                             psq[:, 512 + lo:512 + lo + 128], mask128[:])
                    pt = pt_p.tile([128, 1024], BF16, tag="pt", name="pt")
                    nc.scalar.activation(pt[:], psq[:],
                                         mybir.ActivationFunctionType.Exp, scale=0.125)
                    if lo > 0:
                        nc.vector.memset(pt[:, 0:lo], 0.0)
                        nc.vector.memset(pt[:, 512:512 + lo], 0.0)
                    pts[kc] = pt
                for kc in range(r0, r0 + 4):
                    nc.tensor.matmul(poA[:], vplus[b][kc][:], pts[kc][:, 0:512],
                                     start=(kc == 0), stop=(kc == nk - 1))
                    nc.tensor.matmul(poB[:], vplus[b][kc][:], pts[kc][:, 512:1024],
                                     start=(kc == 0), stop=(kc == nk - 1))
            # normalize: reciprocal of the ones-row, DRAM hop for the
            # partition-broadcast, then scale + emit to the A2A buffer.
            bi = buf_in_a if t == 0 else buf_in_b
            for hh, po in ((0, poA), (1, poB)):
                rc = nrm_p.tile([65, 512], F32, tag="rc", name="rc", bufs=4)
                nc.vector.reciprocal(rc[64:65, :], po[64:65, :])
                dr = dram_p.tile([1, 512], F32, tag="dnd", bufs=4, name="dr")
                nc.sync.dma_start(out=dr[:], in_=rc[64:65, :])
                dn = nrm_p.tile([64, 512], F32, tag="dn", name="dn", bufs=4)
                fl = dr[0]
                nc.sync.dma_start(out=dn[:],
                                  in_=bass.AP(tensor=fl.tensor, offset=fl.offset,
                                              ap=[[0, 64]] + list(fl.ap)))
                av = nrm_p.tile([64, 512], BF16, tag="av", name="av")
                nc.vector.tensor_mul(av[:], po[0:64, :], dn[:])
                nc.sync.dma_start(out=bi[b * 4 + qj, hh * 64:hh * 64 + 64, :], in_=av[:])

        # ---- output projection ----
        attn_full = {}
        ec_even = list(range(0, 16, 2))
        ec_odd = list(range(1, 16, 2))

        def emit_attn_full(ecs):
            for ec in ecs:
                t = pt_p.tile([128, 512], BF16, tag="af", bufs=16, name=f"af{ec}")
                bo = buf_out_a if ec % 2 == 0 else buf_out_b
                nc.gpsimd.dma_start(out=t[:], in_=bo[ec // 2, :, :])
                attn_full[ec] = t

        wo_tiles = {}

        def load_wo(dcg, ecs):
            for ec in ecs:
                wt = wst_p.tile([128, 512], BF16, tag="wo", name="wt")
                nc.sync.dma_start(out=wt[:], in_=wo_e[dcg, ec])
                wo_tiles[(dcg, ec)] = wt

        def emit_outproj(dcg, ecs, first, alt=False):
            for j in range(4):           # sequential accumulator chains
                if not alt or j % 2 == 0:
                    pss_out = psOP.tile([128, 512], F32, tag="op", name=f"op{dcg}_{j}")
                else:
                    # tail: attention is done, psQ is idle -- alternate banks
                    # so chain j+1's matmuls overlap chain j's ov copy-out.
                    pss_out = psQ.tile([128, 1024], F32, tag="qk",
                                       name=f"op{dcg}_{j}")[:, 0:512]
                for i, ec in enumerate(ecs):
                    nc.tensor.matmul(pss_out[:],
                                     wo_tiles[(dcg, ec)][:, j * 128:(j + 1) * 128],
                                     attn_full[ec][:],
                                     start=(i == 0), stop=(i == len(ecs) - 1))
                dc = dcg * 4 + j
                ov = oev_p.tile([128, 512], F32, tag="oev", name="ov")
                if alt:
                    # tail only: scalar is idle there and no exp interleaves,
                    # so no ACT table-set thrash; frees the vector engine.
                    nc.scalar.copy(ov[:], pss_out[:])
                else:
                    nc.vector.tensor_copy(ov[:], pss_out[:])
                if first:
                    nc.sync.dma_start(out=out_e[dc * 128:(dc + 1) * 128, :], in_=ov[:])
                else:
                    nc.gpsimd.dma_start(out=out_e[dc * 128:(dc + 1) * 128, :], in_=ov[:],
                                        accum_op=mybir.AluOpType.add)

        # ================= emission schedule =================
        # phase A: batch-0 projections (K pipelines with the x DMA)
        emit_K(0)
        emit_V(0)
        for qc in range(4):
            emit_Q_chunk(0, 0, qc)

        def interleave(groups, steps):
            si = 0
            n = len(groups)
            for gi, g in enumerate(groups):
                attn_pair(*g)
                take = (len(steps) - si) // (n - gi)
                for _ in range(max(0, take)):
                    steps[si]()
                    si += 1
            while si < len(steps):
                steps[si]()
                si += 1

        # phase B: attn(t0, b0) x [Q(b0,t1), xkv1, K(b1), V(b1), Q(b1,t0)]
        stepsB = []
        for qc in range(4):
            stepsB.append(lambda qc=qc: emit_Q_chunk(0, 1, qc))
        for g in range(4):
            stepsB.append(lambda g=g: emit_xkv_g(1, g))
        stepsB.append(lambda: emit_K(1))
        stepsB.append(lambda: emit_V(1))
        for qc in range(4):
            stepsB.append(lambda qc=qc: emit_Q_chunk(1, 0, qc))
        interleave([(0, 0, qj) for qj in range(4)], stepsB)

        # phase C: attn(t0, b1) x [Q(b1,t1)]
        stepsC = [lambda qc=qc: emit_Q_chunk(1, 1, qc) for qc in range(4)]
        interleave([(0, 1, qj) for qj in range(4)], stepsC)

        nc.gpsimd.collective_compute(
            "AllToAll", mybir.AluOpType.bypass,
            ins=[buf_in_a.opt()], outs=[buf_out_a.opt()],
            replica_groups=[[0, 1, 2, 3, 4, 5, 6, 7]],
        )

        # phase D: attn(t1, b0) x [attn_full evens, wo loads]
        stepsD = [lambda: (emit_attn_full(ec_even), load_wo(0, ec_even),
                           load_wo(1, ec_even))]
        interleave([(1, 0, qj) for qj in range(4)], stepsD)

        # phase E: attn(t1, b1) x [wo loads]; ALL outproj evens are deferred
        # past the A2A#2 emission so the PE has ~32us of A2A-independent work
        # covering both the pre-trigger gap and the collective itself.
        stepsE = [lambda: (load_wo(2, ec_even), load_wo(3, ec_even))]
        interleave([(1, 1, qj) for qj in range(4)], stepsE)

        nc.gpsimd.collective_compute(
            "AllToAll", mybir.AluOpType.bypass,
            ins=[buf_in_b.opt()], outs=[buf_out_b.opt()],
            replica_groups=[[0, 1, 2, 3, 4, 5, 6, 7]],
        )

        # tail: all evens fill the pre-trigger gap + A2A#2 wait; then odds
        for dcg in range(4):
            emit_outproj(dcg, ec_even, True, alt=True)
        emit_attn_full(ec_odd)
        for dcg in range(4):
            load_wo(dcg, ec_odd)
            emit_outproj(dcg, ec_odd, False, alt=True)

    nc.compile()
    return nc


def kernel(x, freqs_cos, freqs_sin, wq, wk, wv, wo, attn_mask):
    x = np.asarray(x, dtype=np.float32)
    freqs_cos = np.asarray(freqs_cos, dtype=np.float32)
    freqs_sin = np.asarray(freqs_sin, dtype=np.float32)
    wq = np.asarray(wq, dtype=np.float32)
    wk = np.asarray(wk, dtype=np.float32)
    wv = np.asarray(wv, dtype=np.float32)
    wo = np.asarray(wo, dtype=np.float32)
    attn_mask = np.asarray(attn_mask)
    assert np.all(attn_mask == 1), "kernel specialized for all-ones attn_mask"

    if "nc" not in _CACHE:
        _CACHE["nc"] = _build()
    nc = _CACHE["nc"]

    idx = np.arange(128)
    i_of_p = (idx % 64) // 2
    ropec = np.ascontiguousarray(freqs_cos.T[i_of_p].astype(np.float32))
    sgn = np.where(idx % 2 == 1, 1.0, -1.0).astype(np.float32)
    ropeg = np.ascontiguousarray((freqs_sin.T[i_of_p] * sgn[:, None]).astype(np.float32))
    perm = np.zeros((128, 128), np.float32)
    perm[idx, idx ^ 1] = 1.0
    m128 = np.where(np.arange(128)[:, None] > np.arange(128)[None, :],
                    np.float32(BF16_MIN), np.float32(0.0)).astype(BD)

    woT = np.ascontiguousarray(wo.T.astype(BD))
    wot = np.ascontiguousarray(woT.reshape(16, 128, 4, 512).transpose(2, 0, 1, 3))
    # xkv: [b, g, p, j, s] with x[b].T row (g*4+j)*128+p
    xT = x.transpose(0, 2, 1).astype(BD)                        # [b, d, s]
    xkv = np.ascontiguousarray(
        xT.reshape(2, 4, 4, 128, S).transpose(0, 1, 3, 2, 4))   # [b, g, p, j, s]

    in_maps = []
    for c in range(NCORES):
        wqr = wq[256 * c:256 * (c + 1)]
        wqt = np.ascontiguousarray(
            wqr.T.astype(BD).reshape(16, 128, 2, 128).transpose(2, 0, 1, 3))
        wkt = np.ascontiguousarray(wk[64 * c:64 * (c + 1)].T.astype(BD).reshape(16, 128, 64))
        wvt = np.ascontiguousarray(wv[64 * c:64 * (c + 1)].T.astype(BD).reshape(16, 128, 64))
        in_maps.append({
            "xkv": xkv, "wqt": wqt, "wkt": wkt, "wvt": wvt, "wot": wot,
            "ropec": ropec, "ropeg": ropeg, "perm": perm, "mask128": m128,
        })

    res = run_bass_kernel_spmd(nc, in_maps, core_ids=list(range(NCORES)))
    _CACHE["last_res"] = res

    out = np.empty((B, S, D), np.float32)
    for c in range(NCORES):
        b, r = c // 4, c % 4
        out[b, 512 * r:512 * (r + 1), :] = res.results[c]["out"].T
    return out
